# revision 9
# baseline (speedup 1.0000x reference)
"""Causal self-attention (B=2, T=2048, C=1024, H=16, Dh=64) on 8 TRN2 cores.

Sharding: data-parallel over B (2) x tensor-parallel over heads (4 groups of
4 heads) = 8 shards. Core i handles batch i//4, heads 4*(i%4)..4*(i%4)+3.
Host pre-marshals each shard's operands (slice + transpose to contraction-
major + cast to bf16, standard tensor-parallel weight layout); each core
computes its QKV projection, causal-softmax attention for its 4 heads, and
its partial out-projection. Host sums the 4 partials per batch (row-parallel
out-projection reduce).

Device program (per core, all matmuls bf16 with f32 PSUM accumulation):
  xt  [1024, 2048] bf16 = x[b].T
  wt  [1024, 768]  bf16 = Wqkv_shard.T   (f = Qp0|Qp1|Kp0|Kp1|V)
  wot [256, 1024]  bf16 = Wout[:, cols].T
  y   [2048, 1024] f32 partial output

  1. qkT[f, t] = sum_c wt[c, f] xt[c, t]      (Q^T, K^T head-pair tiles)
  2. v[t, f]   = sum_c xt[c, t] wt[c, 512+f]  (V tiles + ones column)
  3. per head pair (row-paired K=64 matmuls at partitions 0/64):
       ST[k, q] = exp(0.125 * sum_d K^T[d, k] Q^T[d, q]) (causal-masked)
       outT[d', q] += V[k, d'] ST[k, q]   (d'=65: ones col accumulates Z)
       OUTT[c', q] = outT[c', q] * (1/Z[q])
  4. y[t, f] = sum_c' OUTT[c', t] wot[c', f]
"""

import sys

for _p in ("/opt/trn_rl_repo",):
    if _p not in sys.path:
        sys.path.append(_p)

import numpy as np
import ml_dtypes
from contextlib import ExitStack

import concourse.bass as bass
import concourse.bacc as bacc
import concourse.mybir as mybir
import concourse.tile as tile
from concourse.bass_utils import run_bass_kernel_spmd
from concourse.masks import make_upper_triangular

BF16 = mybir.dt.bfloat16
F32 = mybir.dt.float32
AF = mybir.ActivationFunctionType

T = 2048
C = 1024
N_CORES = 8

_cached_nc = None


def build_program():
    global _cached_nc
    if _cached_nc is not None:
        return _cached_nc
    nc = bacc.Bacc("TRN2", target_bir_lowering=False, debug=False,
                   num_devices=N_CORES)
    xt_d = nc.dram_tensor("xt", [C, T], BF16, kind="ExternalInput").ap()
    wt_d = nc.dram_tensor("wt", [C, 768], BF16, kind="ExternalInput").ap()
    wot_d = nc.dram_tensor("wot", [256, C], BF16, kind="ExternalInput").ap()
    y_d = nc.dram_tensor("y", [T, C], F32, kind="ExternalOutput").ap()

    with tile.TileContext(nc) as tc, ExitStack() as ctx:
        const = ctx.enter_context(tc.tile_pool(name="const", bufs=1))
        sb = ctx.enter_context(tc.tile_pool(name="sb", bufs=1))
        wk = ctx.enter_context(tc.tile_pool(name="wk", bufs=1))
        ps = ctx.enter_context(tc.tile_pool(name="ps", bufs=1, space="PSUM"))

        trimask = const.tile([128, 128], BF16, tag="trimask")
        make_upper_triangular(nc, trimask[:], val=1.0, diag=True)
        zbias = const.tile([128, 1], F32, tag="zbias")
        nc.vector.memset(zbias[:], 0.0)

        XT = [sb.tile([128, T], BF16, tag=f"xt{k}", name=f"xts{k}")
              for k in range(8)]
        WT = [sb.tile([128, 768], BF16, tag=f"wt{k}", name=f"wts{k}")
              for k in range(8)]
        WOT = [sb.tile([128, C], BF16, tag=f"wot{k}", name=f"wots{k}")
               for k in range(2)]
        QT = [sb.tile([128, T], BF16, tag=f"qt{p}", name=f"qts{p}")
              for p in range(2)]
        KT = [sb.tile([128, T], BF16, tag=f"kt{p}", name=f"kts{p}")
              for p in range(2)]
        V = [sb.tile([128, 4 * 65], BF16, tag=f"v{t}", name=f"vs{t}")
             for t in range(16)]
        OUTT = [sb.tile([128, T], BF16, tag=f"outt{p}", name=f"outts{p}")
                for p in range(2)]

        # interleave so the first QK accumulation can start after ~2 tiles
        for k in range(8):
            nc.sync.dma_start(XT[k][:], xt_d[128 * k:128 * (k + 1), :])
            nc.sync.dma_start(WT[k][:], wt_d[128 * k:128 * (k + 1), :])
        for k in range(2):
            nc.sync.dma_start(WOT[k][:], wot_d[128 * k:128 * (k + 1), :])

        # PSUM budget (8 banks): "st" [128,1024]x2 = 4 (attention scores),
        # "pv" [128,1024]x1 = 2 (attention out), "pj" [128,512]x2 = 2
        # (projections + out-projection). Boundary filler matmuls on "pj"
        # cover the single-buffered "pv" release window.

        def proj_qk_group(p, which, nb):
            # one psum accumulation group: QT/KT[p], t-block nb
            dst = QT[p] if which == 0 else KT[p]
            fb = p * 128 + (0 if which == 0 else 256)
            pj = ps.tile([128, 512], F32, tag="pj", bufs=2, name="pj")
            for kc in range(8):
                nc.tensor.matmul(
                    pj[:],
                    WT[kc][:, fb:fb + 128],
                    XT[kc][:, nb * 512:(nb + 1) * 512],
                    start=(kc == 0), stop=(kc == 7))
            nc.vector.tensor_copy(dst[:, nb * 512:(nb + 1) * 512], pj[:])

        def proj_v_tt(tt):
            pj = ps.tile([128, 512], F32, tag="pj", bufs=2, name="pj")
            for kc in range(8):
                nc.tensor.matmul(
                    pj[:, 0:256],
                    XT[kc][:, tt * 128:(tt + 1) * 128],
                    WT[kc][:, 512:768],
                    start=(kc == 0), stop=(kc == 7))
            vv = V[tt].rearrange("p (h e) -> p h e", e=65)
            nc.vector.tensor_copy(
                vv[:, :, 0:64],
                pj[:, 0:256].rearrange("p (h e) -> p h e", e=64))
            nc.vector.memset(vv[:, :, 64:65], 1.0)

        def attention_qb(p, qb):
            hA, hB = 2 * p, 2 * p + 1
            # merged A/B psum: head A in cols 0:512, head B in 512:1024
            pv = ps.tile([128, 1024], F32, tag="pv", bufs=1, name="pv")
            nkt = (qb + 1) * 4
            for kt in range(nkt):
                off = max(0, kt * 128 - qb * 512)
                ncols = 512 - off
                qs = qb * 512 + off
                stp = ps.tile([128, 1024], F32, tag="st", bufs=2, name="stp")
                nc.tensor.matmul(
                    stp[:, 0:ncols],
                    KT[p][0:64, kt * 128:(kt + 1) * 128],
                    QT[p][0:64, qs:qs + ncols],
                    start=True, stop=True)
                nc.tensor.matmul(
                    stp[:, 512:512 + ncols],
                    KT[p][64:128, kt * 128:(kt + 1) * 128],
                    QT[p][64:128, qs:qs + ncols],
                    start=True, stop=True)
                sa = wk.tile([128, 1024], BF16, tag="sa_sb", bufs=3, name="sa")
                nc.scalar.activation(
                    sa.rearrange("p (g n) -> p g n", g=2)[:, :, 0:ncols],
                    stp.rearrange("p (g n) -> p g n", g=2)[:, :, 0:ncols],
                    AF.Exp, bias=zbias[:], scale=0.125)
                if off > 0 or kt * 128 == qb * 512:
                    m3 = sa.rearrange("p (g n) -> p g n", g=2)[:, :, 0:128]
                    nc.vector.tensor_mul(
                        m3, m3,
                        trimask[:].unsqueeze(1).broadcast_to([128, 2, 128]))
                nc.tensor.matmul(
                    pv[0:65, off:512],
                    V[kt][:, hA * 65:hA * 65 + 65],
                    sa[:, 0:ncols],
                    start=(kt == 0), stop=(kt == nkt - 1))
                nc.tensor.matmul(
                    pv[0:65, 512 + off:1024],
                    V[kt][:, hB * 65:hB * 65 + 65],
                    sa[:, 512:512 + ncols],
                    start=(kt == 0), stop=(kt == nkt - 1))
            # evict unnormalized out + Z; 1/Z via DMA-reshape so the
            # reciprocal runs on 128 DVE lanes instead of one
            u = wk.tile([65, 1024], F32, tag="u", bufs=2, name="u")
            nc.vector.tensor_copy(u[:], pv[0:65, :])
            zcol = wk.tile([128, 8], F32, tag="zcol", bufs=2, name="zcol")
            nc.sync.dma_start(zcol[:], u[64:65, :])
            nc.vector.reciprocal(zcol[:], zcol[:])
            zrow = wk.tile([1, 1024], F32, tag="zrow", bufs=2, name="zrow")
            nc.sync.dma_start(zrow[:], zcol[:])
            zb = wk.tile([64, 1024], F32, tag="zb", bufs=2, name="zb")
            nc.gpsimd.partition_broadcast(zb[:], zrow[:])
            qsl = slice(qb * 512, (qb + 1) * 512)
            nc.vector.tensor_mul(OUTT[p][0:64, qsl], u[0:64, 0:512],
                                 zb[:, 0:512])
            nc.vector.tensor_mul(OUTT[p][64:128, qsl], u[0:64, 512:1024],
                                 zb[:, 512:1024])

        def outproj_tt(tt):
            ysb = wk.tile([128, C], F32, tag="ysb", bufs=2, name="ysb")
            for fb in range(2):
                pj = ps.tile([128, 512], F32, tag="pj", bufs=2, name="pj")
                for kcp in range(2):
                    nc.tensor.matmul(
                        pj[:],
                        OUTT[kcp][:, tt * 128:(tt + 1) * 128],
                        WOT[kcp][:, fb * 512:(fb + 1) * 512],
                        start=(kcp == 0), stop=(kcp == 1))
                nc.vector.tensor_copy(ysb[:, fb * 512:(fb + 1) * 512], pj[:])
            nc.sync.dma_start(y_d[tt * 128:(tt + 1) * 128, :], ysb[:])

        # Emission order = PE execution order (PE is in-order): interleave
        # independent projection / out-projection matmuls into the ACT-bound
        # attention phases so the PE never idles long enough to cool.
        # Emission order = PE order. Prefix is QK0 only; V-projection and QK1
        # ride as PE filler inside the ACT-paced attention-p0 windows, and
        # the out-projection rides inside attention-p1 (one q-block late so
        # the normalize chain never stalls the in-order PE).
        for which in range(2):
            for nb in range(4):
                proj_qk_group(0, which, nb)
        qk1 = [(1, w, nb) for w in range(2) for nb in range(4)]
        for qb in range(4):
            for tt in range(4 * qb, 4 * qb + 4):
                proj_v_tt(tt)
            attention_qb(0, qb)
            for _ in range(2):
                if qk1:
                    proj_qk_group(*qk1.pop(0))
        out_sched = {1: [0, 1, 2], 2: [3, 4, 5, 6, 7], 3: [8, 9, 10, 11, 12, 13]}
        for qb in range(4):
            attention_qb(1, qb)
            for tt in out_sched.get(qb, []):
                outproj_tt(tt)
        for tt in (14, 15):
            outproj_tt(tt)

    nc.compile()
    _cached_nc = nc
    return nc


def shard_inputs(x, Wqkv, Wout):
    """Full inputs -> 8 per-core input dicts (sliced/transposed/bf16-cast)."""
    bf = ml_dtypes.bfloat16
    in_maps = []
    for i in range(N_CORES):
        b, g = divmod(i, 4)
        r = slice(256 * g, 256 * (g + 1))
        w_my = np.concatenate(
            [Wqkv[0:1024][r], Wqkv[1024:2048][r], Wqkv[2048:3072][r]], axis=0)
        in_maps.append({
            "xt": np.ascontiguousarray(x[b].T).astype(bf),
            "wt": np.ascontiguousarray(w_my.T).astype(bf),
            "wot": np.ascontiguousarray(Wout[:, r].T).astype(bf),
        })
    return in_maps


def gather_output(results):
    """8 per-core partial y -> full [2, T, C] f32 output."""
    y = np.zeros((2, T, C), dtype=np.float64)
    for i in range(N_CORES):
        y[i // 4] += np.asarray(results[i]["y"], dtype=np.float64)
    return y.astype(np.float32)


def kernel(x, Wqkv, Wout):
    x = np.asarray(x)
    Wqkv = np.asarray(Wqkv)
    Wout = np.asarray(Wout)
    nc = build_program()
    in_maps = shard_inputs(x, Wqkv, Wout)
    res = run_bass_kernel_spmd(nc, in_maps, core_ids=list(range(N_CORES)))
    return gather_output(res.results)


# revision 11
# speedup vs baseline: 1.0149x; 1.0149x over previous
"""Causal self-attention (B=2, T=2048, C=1024, H=16, Dh=64) on 8 TRN2 cores.

Sharding: data-parallel over B (2) x tensor-parallel over heads (4 groups of
4 heads) = 8 shards. Core i handles batch i//4, heads 4*(i%4)..4*(i%4)+3.
Host pre-marshals each shard's operands (slice + transpose to contraction-
major + cast to bf16, standard tensor-parallel weight layout); each core
computes its QKV projection, causal-softmax attention for its 4 heads, and
its partial out-projection. Host sums the 4 partials per batch (row-parallel
out-projection reduce).

Device program (per core, all matmuls bf16 with f32 PSUM accumulation):
  xt  [1024, 2048] bf16 = x[b].T
  wt  [1024, 768]  bf16 = Wqkv_shard.T   (f = Qp0|Qp1|Kp0|Kp1|V)
  wot [256, 1024]  bf16 = Wout[:, cols].T
  y   [2048, 1024] f32 partial output

  1. qkT[f, t] = sum_c wt[c, f] xt[c, t]      (Q^T, K^T head-pair tiles)
  2. v[t, f]   = sum_c xt[c, t] wt[c, 512+f]  (V tiles + ones column)
  3. per head pair (row-paired K=64 matmuls at partitions 0/64):
       ST[k, q] = exp(0.125 * sum_d K^T[d, k] Q^T[d, q]) (causal-masked)
       outT[d', q] += V[k, d'] ST[k, q]   (d'=65: ones col accumulates Z)
       OUTT[c', q] = outT[c', q] * (1/Z[q])
  4. y[t, f] = sum_c' OUTT[c', t] wot[c', f]
"""

import sys

for _p in ("/opt/trn_rl_repo",):
    if _p not in sys.path:
        sys.path.append(_p)

import numpy as np
import ml_dtypes
from contextlib import ExitStack

import concourse.bass as bass
import concourse.bacc as bacc
import concourse.mybir as mybir
import concourse.tile as tile
from concourse.bass_utils import run_bass_kernel_spmd
from concourse.masks import make_upper_triangular

BF16 = mybir.dt.bfloat16
F32 = mybir.dt.float32
AF = mybir.ActivationFunctionType

T = 2048
C = 1024
N_CORES = 8

_cached_nc = None


def build_program():
    global _cached_nc
    if _cached_nc is not None:
        return _cached_nc
    nc = bacc.Bacc("TRN2", target_bir_lowering=False, debug=False,
                   num_devices=N_CORES)
    xt_d = nc.dram_tensor("xt", [C, T], BF16, kind="ExternalInput").ap()
    wt_d = nc.dram_tensor("wt", [C, 768], BF16, kind="ExternalInput").ap()
    wot_d = nc.dram_tensor("wot", [256, C], BF16, kind="ExternalInput").ap()
    y_d = nc.dram_tensor("y", [T, C], F32, kind="ExternalOutput").ap()

    with tile.TileContext(nc) as tc, ExitStack() as ctx:
        const = ctx.enter_context(tc.tile_pool(name="const", bufs=1))
        sb = ctx.enter_context(tc.tile_pool(name="sb", bufs=1))
        wk = ctx.enter_context(tc.tile_pool(name="wk", bufs=1))
        ps = ctx.enter_context(tc.tile_pool(name="ps", bufs=1, space="PSUM"))

        trimask = const.tile([128, 128], BF16, tag="trimask")
        make_upper_triangular(nc, trimask[:], val=1.0, diag=True)
        zbias = const.tile([128, 1], F32, tag="zbias")
        nc.vector.memset(zbias[:], 0.0)

        XT = [sb.tile([128, T], BF16, tag=f"xt{k}", name=f"xts{k}")
              for k in range(8)]
        WT = [sb.tile([128, 768], BF16, tag=f"wt{k}", name=f"wts{k}")
              for k in range(8)]
        WOT = [sb.tile([128, C], BF16, tag=f"wot{k}", name=f"wots{k}")
               for k in range(2)]
        QT = [sb.tile([128, T], BF16, tag=f"qt{p}", name=f"qts{p}")
              for p in range(2)]
        KT = [sb.tile([128, T], BF16, tag=f"kt{p}", name=f"kts{p}")
              for p in range(2)]
        V = [sb.tile([128, 4 * 65], BF16, tag=f"v{t}", name=f"vs{t}")
             for t in range(16)]
        OUTT = [sb.tile([128, T], BF16, tag=f"outt{p}", name=f"outts{p}")
                for p in range(2)]

        # two HWDGE queues in parallel so the first QK accumulation group
        # has its operands after ~2 tile-loads
        for k in range(8):
            nc.sync.dma_start(XT[k][:], xt_d[128 * k:128 * (k + 1), :])
            nc.scalar.dma_start(WT[k][:], wt_d[128 * k:128 * (k + 1), :])
        for k in range(2):
            nc.scalar.dma_start(WOT[k][:], wot_d[128 * k:128 * (k + 1), :])

        # PSUM budget (8 banks): "st" [128,1024]x2 = 4 (attention scores),
        # "pv" [128,1024]x1 = 2 (attention out), "pj" [128,512]x2 = 2
        # (projections + out-projection). Boundary filler matmuls on "pj"
        # cover the single-buffered "pv" release window.

        def proj_qk_group(p, which, nb):
            # one psum accumulation group: QT/KT[p], t-block nb
            dst = QT[p] if which == 0 else KT[p]
            fb = p * 128 + (0 if which == 0 else 256)
            pj = ps.tile([128, 512], F32, tag="pj", bufs=2, name="pj")
            for kc in range(8):
                nc.tensor.matmul(
                    pj[:],
                    WT[kc][:, fb:fb + 128],
                    XT[kc][:, nb * 512:(nb + 1) * 512],
                    start=(kc == 0), stop=(kc == 7))
            nc.vector.tensor_copy(dst[:, nb * 512:(nb + 1) * 512], pj[:])

        def proj_v_tt(tt):
            pj = ps.tile([128, 512], F32, tag="pj", bufs=2, name="pj")
            for kc in range(8):
                nc.tensor.matmul(
                    pj[:, 0:256],
                    XT[kc][:, tt * 128:(tt + 1) * 128],
                    WT[kc][:, 512:768],
                    start=(kc == 0), stop=(kc == 7))
            vv = V[tt].rearrange("p (h e) -> p h e", e=65)
            nc.vector.tensor_copy(
                vv[:, :, 0:64],
                pj[:, 0:256].rearrange("p (h e) -> p h e", e=64))
            nc.vector.memset(vv[:, :, 64:65], 1.0)

        def attention_qb(p, qb):
            hA, hB = 2 * p, 2 * p + 1
            # merged A/B psum: head A in cols 0:512, head B in 512:1024
            pv = ps.tile([128, 1024], F32, tag="pv", bufs=1, name="pv")
            nkt = (qb + 1) * 4

            def emit_pv(kt, sa, off, ncols):
                nc.tensor.matmul(
                    pv[0:65, off:512],
                    V[kt][:, hA * 65:hA * 65 + 65],
                    sa[:, 0:ncols],
                    start=(kt == 0), stop=(kt == nkt - 1))
                nc.tensor.matmul(
                    pv[0:65, 512 + off:1024],
                    V[kt][:, hB * 65:hB * 65 + 65],
                    sa[:, 512:512 + ncols],
                    start=(kt == 0), stop=(kt == nkt - 1))

            # software-pipelined: PV(kt) is emitted after ST(kt+1) so the
            # in-order PE never waits on the exp of the current iteration
            pending = None
            for kt in range(nkt):
                off = max(0, kt * 128 - qb * 512)
                ncols = 512 - off
                qs = qb * 512 + off
                stp = ps.tile([128, 1024], F32, tag="st", bufs=2, name="stp")
                nc.tensor.matmul(
                    stp[:, 0:ncols],
                    KT[p][0:64, kt * 128:(kt + 1) * 128],
                    QT[p][0:64, qs:qs + ncols],
                    start=True, stop=True)
                nc.tensor.matmul(
                    stp[:, 512:512 + ncols],
                    KT[p][64:128, kt * 128:(kt + 1) * 128],
                    QT[p][64:128, qs:qs + ncols],
                    start=True, stop=True)
                sa = wk.tile([128, 1024], BF16, tag="sa_sb", bufs=3, name="sa")
                nc.scalar.activation(
                    sa.rearrange("p (g n) -> p g n", g=2)[:, :, 0:ncols],
                    stp.rearrange("p (g n) -> p g n", g=2)[:, :, 0:ncols],
                    AF.Exp, bias=zbias[:], scale=0.125)
                if off > 0 or kt * 128 == qb * 512:
                    m3 = sa.rearrange("p (g n) -> p g n", g=2)[:, :, 0:128]
                    nc.vector.tensor_mul(
                        m3, m3,
                        trimask[:].unsqueeze(1).broadcast_to([128, 2, 128]))
                if pending is not None:
                    emit_pv(*pending)
                pending = (kt, sa, off, ncols)
            emit_pv(*pending)
            # evict unnormalized out + Z; 1/Z via DMA-reshape so the
            # reciprocal runs on 128 DVE lanes instead of one
            u = wk.tile([65, 1024], F32, tag="u", bufs=2, name="u")
            nc.vector.tensor_copy(u[:], pv[0:65, :])
            zcol = wk.tile([128, 8], F32, tag="zcol", bufs=2, name="zcol")
            nc.sync.dma_start(zcol[:], u[64:65, :])
            nc.vector.reciprocal(zcol[:], zcol[:])
            zrow = wk.tile([1, 1024], F32, tag="zrow", bufs=2, name="zrow")
            nc.sync.dma_start(zrow[:], zcol[:])
            zb = wk.tile([64, 1024], F32, tag="zb", bufs=2, name="zb")
            nc.gpsimd.partition_broadcast(zb[:], zrow[:])
            qsl = slice(qb * 512, (qb + 1) * 512)
            nc.vector.tensor_mul(OUTT[p][0:64, qsl], u[0:64, 0:512],
                                 zb[:, 0:512])
            nc.vector.tensor_mul(OUTT[p][64:128, qsl], u[0:64, 512:1024],
                                 zb[:, 512:1024])

        def outproj_tt(tt):
            ysb = wk.tile([128, C], F32, tag="ysb", bufs=2, name="ysb")
            for fb in range(2):
                pj = ps.tile([128, 512], F32, tag="pj", bufs=2, name="pj")
                for kcp in range(2):
                    nc.tensor.matmul(
                        pj[:],
                        OUTT[kcp][:, tt * 128:(tt + 1) * 128],
                        WOT[kcp][:, fb * 512:(fb + 1) * 512],
                        start=(kcp == 0), stop=(kcp == 1))
                nc.vector.tensor_copy(ysb[:, fb * 512:(fb + 1) * 512], pj[:])
            nc.sync.dma_start(y_d[tt * 128:(tt + 1) * 128, :], ysb[:])

        # Emission order = PE execution order (PE is in-order): interleave
        # independent projection / out-projection matmuls into the ACT-bound
        # attention phases so the PE never idles long enough to cool.
        # Emission order = PE order. Prefix is QK0 only; V-projection and QK1
        # ride as PE filler inside the ACT-paced attention-p0 windows, and
        # the out-projection rides inside attention-p1 (one q-block late so
        # the normalize chain never stalls the in-order PE).
        for which in range(2):
            for nb in range(4):
                proj_qk_group(0, which, nb)
        qk1 = [(1, w, nb) for w in range(2) for nb in range(4)]
        for qb in range(4):
            for tt in range(4 * qb, 4 * qb + 4):
                proj_v_tt(tt)
            attention_qb(0, qb)
            for _ in range(2):
                if qk1:
                    proj_qk_group(*qk1.pop(0))
        out_sched = {1: [0, 1, 2], 2: [3, 4, 5, 6, 7], 3: [8, 9, 10, 11, 12, 13]}
        for qb in range(4):
            attention_qb(1, qb)
            for tt in out_sched.get(qb, []):
                outproj_tt(tt)
        for tt in (14, 15):
            outproj_tt(tt)

    nc.compile()
    _cached_nc = nc
    return nc


def shard_inputs(x, Wqkv, Wout):
    """Full inputs -> 8 per-core input dicts (sliced/transposed/bf16-cast)."""
    bf = ml_dtypes.bfloat16
    in_maps = []
    for i in range(N_CORES):
        b, g = divmod(i, 4)
        r = slice(256 * g, 256 * (g + 1))
        w_my = np.concatenate(
            [Wqkv[0:1024][r], Wqkv[1024:2048][r], Wqkv[2048:3072][r]], axis=0)
        in_maps.append({
            "xt": np.ascontiguousarray(x[b].T).astype(bf),
            "wt": np.ascontiguousarray(w_my.T).astype(bf),
            "wot": np.ascontiguousarray(Wout[:, r].T).astype(bf),
        })
    return in_maps


def gather_output(results):
    """8 per-core partial y -> full [2, T, C] f32 output."""
    y = np.zeros((2, T, C), dtype=np.float64)
    for i in range(N_CORES):
        y[i // 4] += np.asarray(results[i]["y"], dtype=np.float64)
    return y.astype(np.float32)


def kernel(x, Wqkv, Wout):
    x = np.asarray(x)
    Wqkv = np.asarray(Wqkv)
    Wout = np.asarray(Wout)
    nc = build_program()
    in_maps = shard_inputs(x, Wqkv, Wout)
    res = run_bass_kernel_spmd(nc, in_maps, core_ids=list(range(N_CORES)))
    return gather_output(res.results)
